# revision 11
# baseline (speedup 1.0000x reference)
"""Trainium2 Bass kernel for PVT-style spatial-reduction attention with LoRA.

Sharding: 8 cores = (batch b in {0,1}) x (head-pair p in {0..3}); each core
computes 2 of the 16 (b, head) units end-to-end plus a partial projection;
the host sums the 4 partial projections per batch.

All activations live transposed ([feature, token]) on device so every matmul
contraction sits on the partition dim. Host folds: LoRA adapters into the
dense weights, softmax scale into Wq/bq, LayerNorm gamma/beta into Wk/Wv and
the final bias, the (softmax-invariant) k bias is dropped, and the v bias
becomes part of the output bias. The softmax denominator is produced by an
extra all-ones column appended to the stationary V operand, so no extra PE
pass is needed; max-subtraction is skipped (logits are bounded ~|1.8|).
"""
import sys
for _p in ('/opt/trn_rl_repo', '/root/.axon_site/_ro/trn_rl_repo'):
    if _p not in sys.path:
        sys.path.insert(0, _p)

import numpy as np

B, N, C, HEAD, SR, R = 2, 4096, 512, 8, 2, 8
HH = WW = 64
DH = C // HEAD               # 64
M = (HH // SR) * (WW // SR)  # 1024 kv positions
LN_EPS = 1e-5
NCORES = 8

_cached = {}


def _build_nc(reps=1):
    from concourse import bacc, tile, mybir
    import concourse.bass as bass_mod

    f32 = mybir.dt.float32
    f32r = mybir.dt.float32r
    ACT = mybir.ActivationFunctionType

    nc = bacc.Bacc("TRN2", target_bir_lowering=False, debug=False,
                   num_devices=NCORES)
    xT_d = nc.dram_tensor("xT", [4, 128, N], f32r, kind="ExternalInput")
    wsr_d = nc.dram_tensor("wsr", [16, 128, C], f32r, kind="ExternalInput")
    wq_d = nc.dram_tensor("wq", [4, 128, 128], f32r, kind="ExternalInput")
    wk_d = nc.dram_tensor("wk", [4, 128, 128], f32r, kind="ExternalInput")
    wv_d = nc.dram_tensor("wv", [4, 128, 128], f32r, kind="ExternalInput")
    wp_d = nc.dram_tensor("wp", [2, 64, C], f32r, kind="ExternalInput")
    bq_d = nc.dram_tensor("bq", [128, 1], f32, kind="ExternalInput")
    bsr_d = nc.dram_tensor("bsr", [4, 128], f32, kind="ExternalInput")
    cst_d = nc.dram_tensor("cst", [128, 2], f32r, kind="ExternalInput")
    out_d = nc.dram_tensor("outT", [C, N], f32, kind="ExternalOutput")
    scr_sc_d = nc.dram_tensor("scr_sc", [1, M], f32)
    scr_sh_d = nc.dram_tensor("scr_sh", [1, M], f32)
    scr_rec_d = nc.dram_tensor("scr_rec", [16, 512], f32)

    def emit_rep(tc, rp):
        with tc.tile_pool(name=f"mid{rp}", bufs=1) as mid:
            # tiles that must survive into the attention phase
            wq = mid.tile([128, 4, 128], f32r)
            nc.sync.dma_start(wq[:], wq_d.rearrange("t p n -> p t n"))
            wk = mid.tile([128, 4, 128], f32r)
            nc.sync.dma_start(wk[:], wk_d.rearrange("t p n -> p t n"))
            wv = mid.tile([128, 4, 128], f32r)
            nc.sync.dma_start(wv[:], wv_d.rearrange("t p n -> p t n"))
            wp = mid.tile([64, 2, C], f32r)
            nc.sync.dma_start(wp[:], wp_d.rearrange("h p n -> p h n"))
            bq = mid.tile([128, 1], f32)
            nc.sync.dma_start(bq[:], bq_d[:])
            bsr = mid.tile([128, 4], f32)
            nc.sync.dma_start(bsr[:], bsr_d.rearrange("t p -> p t"))
            cst = mid.tile([128, 2], f32r)
            nc.sync.dma_start(cst[:], cst_d[:])
            ones_invC = cst[:, 0:1]
            eps = mid.tile([1, 1], f32)
            nc.vector.memset(eps[:], LN_EPS)
            qT = mid.tile([128, N], f32r)
            kT = mid.tile([128, M], f32r)
            v = mid.tile([128, 8, 130], f32r)

            with tc.tile_pool(name=f"early{rp}", bufs=1) as early, \
                 tc.tile_pool(name=f"sq{rp}", bufs=2) as sqp, \
                 tc.tile_pool(name=f"stats{rp}", bufs=2) as stp, \
                 tc.tile_pool(name=f"pse{rp}", bufs=2, space="PSUM") as pse, \
                 tc.tile_pool(name=f"pst{rp}", bufs=2, space="PSUM") as pst:

                xT = early.tile([128, 4, N], f32r)
                nc.sync.dma_start(xT[:], xT_d.rearrange("t p n -> p t n"))
                wsr = early.tile([128, 16, C], f32r)
                nc.sync.dma_start(wsr[:], wsr_d.rearrange("g p n -> p g n"))

                # ---- phase 1: conv (spatial reduction) -> xs_pre^T [C, M] ----
                # xz holds xs_pre^T, later overwritten in place by z.
                xz = early.tile([128, 4, M], f32r)
                xview = xT.rearrange("p t (ph a pw b) -> p t ph a pw b",
                                     ph=32, a=2, pw=32, b=2)
                for cot in range(4):
                    for qc in range(2):
                        acc = pse.tile([128, 512], f32, tag="mm")
                        for g in range(16):
                            dydx, ct = g // 4, g % 4
                            dy, dx = dydx // 2, dydx % 2
                            rhs = xview[:, ct, qc * 16:(qc + 1) * 16, dy, :, dx]
                            nc.tensor.matmul(
                                acc[:], wsr[:, g, cot * 128:(cot + 1) * 128],
                                rhs, start=(g == 0), stop=(g == 15))
                        nc.scalar.activation(
                            out=xz[:, cot, qc * 512:(qc + 1) * 512], in_=acc[:],
                            func=ACT.Identity, bias=bsr[:, cot:cot + 1],
                            scale=1.0)

                # ---- phase 2: LayerNorm over C (partition dim) ----
                mean = stp.tile([1, M], f32)
                e2 = stp.tile([1, M], f32)
                for mc in range(2):
                    mps = pst.tile([1, 512], f32, tag="st")
                    for ct in range(4):
                        nc.tensor.matmul(mps[:], ones_invC,
                                         xz[:, ct, mc * 512:(mc + 1) * 512],
                                         start=(ct == 0), stop=(ct == 3))
                    nc.vector.tensor_copy(mean[:, mc * 512:(mc + 1) * 512],
                                          mps[:])
                for mc in range(2):
                    eps_ps = pst.tile([1, 512], f32, tag="st")
                    for ct in range(4):
                        sq = sqp.tile([128, 512], f32r)
                        nc.vector.tensor_mul(
                            sq[:], xz[:, ct, mc * 512:(mc + 1) * 512],
                            xz[:, ct, mc * 512:(mc + 1) * 512])
                        nc.tensor.matmul(eps_ps[:], ones_invC, sq[:],
                                         start=(ct == 0), stop=(ct == 3))
                    nc.vector.tensor_copy(e2[:, mc * 512:(mc + 1) * 512],
                                          eps_ps[:])
                msq = stp.tile([1, M], f32)
                nc.vector.tensor_mul(msq[:], mean[:], mean[:])
                nc.vector.tensor_sub(e2[:], e2[:], msq[:])        # var
                nc.scalar.activation(out=e2[:], in_=e2[:], func=ACT.Sqrt,
                                     bias=eps[:], scale=1.0)
                nc.vector.reciprocal(e2[:], e2[:])                # rstd
                nc.vector.tensor_mul(mean[:], mean[:], e2[:])
                nc.scalar.mul(mean[:], mean[:], -1.0)             # -mu*rstd

                def dram_bcast(dst, scr_ap, nrow):
                    ap = bass_mod.AP(tensor=scr_ap.tensor, offset=scr_ap.offset,
                                     ap=[[0, nrow]] + list(scr_ap.ap[1:]))
                    nc.sync.dma_start(dst, ap)

                nc.sync.dma_start(scr_sc_d[:], e2[:])
                nc.sync.dma_start(scr_sh_d[:], mean[:])
                bc_scale = early.tile([128, M], f32)
                bc_shift = early.tile([128, M], f32)
                dram_bcast(bc_scale[:], scr_sc_d[:], 128)
                dram_bcast(bc_shift[:], scr_sh_d[:], 128)
                for ct in range(4):
                    nc.vector.tensor_mul(xz[:, ct, :], xz[:, ct, :],
                                         bc_scale[:])
                    nc.vector.tensor_add(xz[:, ct, :], xz[:, ct, :],
                                         bc_shift[:])
                # xz now holds z = (xs_pre - mu) * rstd (gamma/beta folded)

                # ---- phase 3: projections ----
                for qc in range(8):
                    qps = pse.tile([128, 512], f32, tag="mm")
                    for ct in range(4):
                        nc.tensor.matmul(qps[:], wq[:, ct, :],
                                         xT[:, ct, qc * 512:(qc + 1) * 512],
                                         start=(ct == 0), stop=(ct == 3))
                    nc.scalar.activation(out=qT[:, qc * 512:(qc + 1) * 512],
                                         in_=qps[:], func=ACT.Identity,
                                         bias=bq[:], scale=1.0)
                for kc in range(2):
                    kps = pse.tile([128, 512], f32, tag="mm")
                    for ct in range(4):
                        nc.tensor.matmul(kps[:], wk[:, ct, :],
                                         xz[:, ct, kc * 512:(kc + 1) * 512],
                                         start=(ct == 0), stop=(ct == 3))
                    nc.vector.tensor_copy(kT[:, kc * 512:(kc + 1) * 512],
                                          kps[:])
                c1 = cst_d[:, 1:2]
                ones_bc = bass_mod.AP(tensor=c1.tensor, offset=c1.offset,
                                      ap=[list(c1.ap[0]), [0, 8], [0, 1]])
                nc.sync.dma_start(v[:, :, 64:65], ones_bc)
                nc.sync.dma_start(v[:, :, 129:130], ones_bc)
                for kt in range(8):
                    vps_full = pse.tile([128, 512], f32, tag="mm", name="vps")
                    vps = vps_full[:, 0:128]
                    for ct in range(4):
                        nc.tensor.matmul(vps[:],
                                         xz[:, ct, kt * 128:(kt + 1) * 128],
                                         wv[:, ct, :],
                                         start=(ct == 0), stop=(ct == 3))
                    vdst = bass_mod.AP(tensor=v.tensor,
                                       offset=v.offset + kt * 130,
                                       ap=[list(v.ap[0]), [65, 2], [1, 64]])
                    nc.vector.tensor_copy(
                        vdst, vps.rearrange("p (h d) -> p h d", h=2))

            # early pools close; attention pools open.
            with tc.tile_pool(name=f"attn{rp}", bufs=1) as attn, \
                 tc.tile_pool(name=f"pexp{rp}", bufs=4) as pexp, \
                 tc.tile_pool(name=f"recb{rp}", bufs=2) as recb, \
                 tc.tile_pool(name=f"obuf{rp}", bufs=3) as obp, \
                 tc.tile_pool(name=f"pss{rp}", bufs=3, space="PSUM") as pss, \
                 tc.tile_pool(name=f"pso{rp}", bufs=2, space="PSUM") as pso, \
                 tc.tile_pool(name=f"psp{rp}", bufs=2, space="PSUM") as psp:

                outT = [attn.tile([65, 8, 512], f32r, tag=f"outT{h}",
                                  name=f"outT{h}") for h in range(2)]
                for qc in range(8):
                    for h in range(2):
                        ops = pso.tile([65, 512], f32)
                        for kt in range(8):
                            sps = pss.tile([128, 512], f32)
                            nc.tensor.matmul(
                                sps[:],
                                kT[64 * h:64 * h + 64, kt * 128:(kt + 1) * 128],
                                qT[64 * h:64 * h + 64,
                                   qc * 512:(qc + 1) * 512],
                                start=True, stop=True)
                            pexp_t = pexp.tile([128, 512], f32r)
                            nc.scalar.activation(out=pexp_t[:], in_=sps[:],
                                                 func=ACT.Exp)
                            nc.tensor.matmul(ops[:],
                                             v[:, kt, 65 * h:65 * h + 65],
                                             pexp_t[:],
                                             start=(kt == 0), stop=(kt == 7))
                        nc.vector.tensor_copy(outT[h][:, qc, :], ops[:])
                        with nc.allow_low_precision(reason="f32r is 4-byte"):
                            nc.vector.reciprocal(outT[h][64:65, qc, :],
                                                 outT[h][64:65, qc, :])
                        nc.sync.dma_start(scr_rec_d[h * 8 + qc, :],
                                          outT[h][64:65, qc, :].bitcast(f32))
                        rb = recb.tile([64, 512], f32r)
                        sr = scr_rec_d[h * 8 + qc:h * 8 + qc + 1, :].bitcast(f32r)
                        ap = bass_mod.AP(tensor=sr.tensor, offset=sr.offset,
                                         ap=[[0, 64]] + list(sr.ap[1:]))
                        nc.sync.dma_start(rb[:], ap)
                        nc.vector.tensor_mul(outT[h][0:64, qc, :],
                                             outT[h][0:64, qc, :], rb[:])
                    for cot in range(4):
                        pps = psp.tile([128, 512], f32)
                        for h in range(2):
                            nc.tensor.matmul(
                                pps[:],
                                wp[0:64, h, cot * 128:(cot + 1) * 128],
                                outT[h][0:64, qc, :],
                                start=(h == 0), stop=(h == 1))
                        ob = obp.tile([128, 512], f32)
                        nc.vector.tensor_copy(ob[:], pps[:])
                        nc.sync.dma_start(
                            out_d[cot * 128:(cot + 1) * 128,
                                  qc * 512:(qc + 1) * 512], ob[:])

    with tile.TileContext(nc) as tc:
        for rp in range(reps):
            emit_rep(tc, rp)

    nc.compile()
    return nc


def _host_prep(inputs):
    x = inputs["x"]; Wq = inputs["Wq"]; bq = inputs["bq"]
    Wkv = inputs["Wkv"]; bkv = inputs["bkv"]
    Wproj = inputs["Wproj"]; bproj = inputs["bproj"]
    Aq = inputs["Aq"]; Bq = inputs["Bq"]; Av = inputs["Av"]; Bv = inputs["Bv"]
    Wsr = inputs["Wsr"]; bsr = inputs["bsr"]
    gamma = inputs["gamma"]; beta = inputs["beta"]
    scale = DH ** -0.5

    Wq_eff = ((Wq + Aq @ Bq) * scale).astype(np.float32)
    bq_eff = (bq * scale).astype(np.float32)
    Wk = Wkv[:, :C]; Wv = Wkv[:, C:]
    AvBv = (Av @ Bv).astype(np.float32)
    Wk_g = (gamma[:, None] * (Wk + AvBv)).astype(np.float32)
    Wv_g = (gamma[:, None] * (Wv + AvBv)).astype(np.float32)
    bv_eff = (beta @ (Wv + AvBv) + bkv[C:]).astype(np.float32)
    bfinal = (bproj + bv_eff @ Wproj).astype(np.float32)
    Wsr_flat = np.ascontiguousarray(Wsr.reshape(4 * C, C), np.float32)

    in_maps = []
    for core in range(NCORES):
        b, p = core // 4, core % 4
        cols = slice(128 * p, 128 * p + 128)
        m = {
            "xT": np.ascontiguousarray(x[b].T).reshape(4, 128, N),
            "wsr": Wsr_flat.reshape(16, 128, C),
            "wq": np.ascontiguousarray(Wq_eff[:, cols]).reshape(4, 128, 128),
            "wk": np.ascontiguousarray(Wk_g[:, cols]).reshape(4, 128, 128),
            "wv": np.ascontiguousarray(Wv_g[:, cols]).reshape(4, 128, 128),
            "wp": np.ascontiguousarray(Wproj[cols, :]).reshape(2, 64, C),
            "bq": np.ascontiguousarray(bq_eff[cols]).reshape(128, 1),
            "bsr": np.ascontiguousarray(bsr).reshape(4, 128),
            "cst": np.stack([np.full(128, 1.0 / C, np.float32),
                             np.ones(128, np.float32)], axis=1),
        }
        in_maps.append({k: np.ascontiguousarray(v, np.float32)
                        for k, v in m.items()})
    return in_maps, bfinal


def run_device(inputs, reps=1):
    from concourse.bass_utils import run_bass_kernel_spmd
    key = f"nc{reps}"
    if key not in _cached:
        _cached[key] = _build_nc(reps)
    nc = _cached[key]
    in_maps, bfinal = _host_prep(inputs)
    res = run_bass_kernel_spmd(nc, in_maps, core_ids=list(range(NCORES)))
    return res, bfinal


def kernel(**inputs):
    inputs = {k: np.asarray(v) for k, v in inputs.items()}
    res, bfinal = run_device(inputs, reps=1)
    out = np.zeros((B, N, C), np.float32)
    for b in range(B):
        acc = np.zeros((C, N), np.float64)
        for p in range(4):
            acc += res.results[4 * b + p]["outT"]
        out[b] = acc.T.astype(np.float32) + bfinal[None, :]
    return out
